# revision 27
# baseline (speedup 1.0000x reference)
"""Trainium2 Bass kernel for nn_AttentionHead (B=8, T=512, V=25, C=128, Dk=Dv=64).

Sharding: data-parallel over batch B across 8 NeuronCores (batch b -> core b).
No cross-device communication.

Per-core design (per vertex v, fp16 datapath, fp32 PSUM accumulation):
- x pre-transposed on host to (V, C, T) fp16; one contiguous DMA per
  vertex pair in, one per pair out (512B+ lines both directions).
- scoresT ([s, t]) layout packs the 4 diagonal 128x128 blocks adjacently at
  cols [0:512), followed by the off-diagonal groups:
  off0 (s-chunk0, t=128..511) at [512:896), off1 at [896:1152),
  off2 at [1152:1280).
- causal masking costs no vector work: a constant strictly-triangular
  -24 matrix is accumulated into the diagonal blocks by one extra
  128->512 matmul (lhsT=trit, rhs=tiled identity); exp(-24+s) < 4e-10
  underflows to 0 in fp16 (and is negligible even in exact arithmetic).
- exp is split across engines: Act table-exps the diagonal segment (all
  masked cols) plus the first ACT_B off-diag cols; DVE approximates exp
  on the never-masked remainder via the Schraudolph bit trick:
  et_i16 = round(s*1477.32 + 15308) bitcast fp16 (rel err ~8e-3).
- value matrix carries an eb=exp(beta) column so the out matmul
  accumulates the softmax denominator for free; bias cross terms
  (alpha_t, beta_s) folded via the augmented wva projection.
- unoccupied slots contribute exp(0)=1: den += (511-t)*exp(-c0-alpha_t),
  via the host cnt table and ea=exp(-alpha).
- elementwise work is paired across two vertices (half the instruction
  overhead). GPSIMD cannot touch PSUM, so all PSUM->SBUF crossings run
  on Act (z copy, diag exp) and DVE (Schraudolph, v4 add, den add,
  normalize); GpSimd handles SBUF-only f16 ops (eb scale, ones column,
  den cnt*ea). PSUM pair tiles pad each half to a full 2KB bank -- an
  accumulating matmul region must not straddle a bank boundary.
"""

import numpy as np
from contextlib import ExitStack

import concourse.bass as bass  # noqa: F401
import concourse.tile as tile
from concourse import bacc, mybir
from concourse.bass_utils import run_bass_kernel_spmd

B, T, V, C = 8, 512, 25, 128
DK, DV = 64, 64
P = 128
NT = T // P  # 4 tiles of 128 along T
N_CORES = 8

# et column layout: 4 diag blocks at [0:512), then off-diag groups:
# off0 (s-chunk0, 384 cols) at [512:896), off2 (128) at [896:1024),
# off1 (256) at [1024:1280). PSUM is processed as three 512-col
# segments: A=[0:512) diag, B=[512:1024) off0+off2, C=[1024:1280) off1.
OFF_BASE = {0: 512, 1: 1024, 2: 896}
SC_TOT = 1280

# Schraudolph fp16 exp constants (tuned numerically: rel err ~7.8e-3)
SCH_C1 = 1024.0 / float(np.log(2.0))
SCH_C2 = 15.0 * 1024.0 - 52.0

# engine split knobs (sim-tuned): Act additionally exps segB[0:ACT_B);
# z-copy cols [0:ZA) on Act, rest on DVE.
ACT_B = 192
ZA = 512

F32 = mybir.dt.float32
F16 = mybir.dt.float16
I16 = mybir.dt.int16
AF = mybir.ActivationFunctionType
ALU = mybir.AluOpType

_PROGRAM_CACHE = {}


def build_program(n_v=V, n_rep=1):
    nc = bacc.Bacc(
        "TRN2", target_bir_lowering=False, debug=False, num_devices=N_CORES
    )
    xt = nc.dram_tensor("xt", [n_v, C, T], F16, kind="ExternalInput").ap()
    am = nc.dram_tensor("am", [C, C], F16, kind="ExternalInput").ap()
    wva = nc.dram_tensor("wva", [C, DV + 2], F16, kind="ExternalInput").ap()
    bvb = nc.dram_tensor("bvb", [P, DV], F32, kind="ExternalInput").ap()
    cnt = nc.dram_tensor("cnt", [P, NT], F32, kind="ExternalInput").ap()
    trit = nc.dram_tensor("trit", [P, P], F16, kind="ExternalInput").ap()
    i4 = nc.dram_tensor("i4", [P, NT * P], F16, kind="ExternalInput").ap()
    out = nc.dram_tensor("out", [n_v, P, NT * DV], F16,
                         kind="ExternalOutput").ap()

    with tile.TileContext(nc) as tc, ExitStack() as ctx:
        consts = ctx.enter_context(tc.tile_pool(name="consts", bufs=1))
        sbx = ctx.enter_context(tc.tile_pool(name="sbx", bufs=3))
        sbqk = ctx.enter_context(tc.tile_pool(name="sbqk", bufs=3))
        sbv = ctx.enter_context(tc.tile_pool(name="sbv", bufs=5))
        sbe = ctx.enter_context(tc.tile_pool(name="sbe", bufs=3))
        sbo = ctx.enter_context(tc.tile_pool(name="sbo", bufs=2))
        sbs = ctx.enter_context(tc.tile_pool(name="sbs", bufs=8))
        psq = ctx.enter_context(tc.tile_pool(name="psq", bufs=1, space="PSUM"))
        psv = ctx.enter_context(tc.tile_pool(name="psv", bufs=1, space="PSUM"))
        pso = ctx.enter_context(tc.tile_pool(name="pso", bufs=1, space="PSUM"))
        pss = ctx.enter_context(tc.tile_pool(name="pss", bufs=1, space="PSUM"))

        am_t = consts.tile([C, C], F16)
        nc.sync.dma_start(am_t[:], am[:])
        wva_t = consts.tile([C, DV + 2], F16)
        nc.sync.dma_start(wva_t[:], wva[:])
        bvb_t = consts.tile([P, DV], F32)
        nc.sync.dma_start(bvb_t[:], bvb[:])
        cnt_t = consts.tile([P, NT], F32)
        nc.sync.dma_start(cnt_t[:], cnt[:])
        trit_t = consts.tile([P, P], F16)
        nc.sync.dma_start(trit_t[:], trit[:])
        i4_t = consts.tile([P, NT * P], F16)
        nc.sync.dma_start(i4_t[:], i4[:])

        for rep in range(n_rep):
          xt_pair = {}
          state = {}
          pstate = {}
          ostate = {}

          def front(v):
            if v % 2 == 0:
                xp = sbx.tile([C, 2, T], F16, tag="xt", name="xp")
                hi = min(2, n_v - v)
                nc.sync.dma_start(
                    xp[:, 0:hi, :],
                    xt[v:v + hi].rearrange("v c t -> c v t"))
                xt_pair[v] = xp
            xt_t = xt_pair[v - v % 2][:, v % 2, :]

            # z = A^T x (A = scale * Wq^T Wk folded on host)
            z_ps = psq.tile([C, T], F32, tag="zp", name="z_ps")
            nc.tensor.matmul(z_ps[:], am_t[:], xt_t, start=True, stop=True)
            zt_sb = sbqk.tile([C, T], F16, tag="zts", name="zt_sb")
            if ZA > 0:
                nc.scalar.activation(zt_sb[:, 0:ZA], z_ps[:, 0:ZA], AF.Copy)
            if ZA < T:
                nc.vector.tensor_copy(zt_sb[:, ZA:T], z_ps[:, ZA:T])

            # v projection into the pair slot: [p, 2, NT*(DV+2)]
            W = DV + 2
            if v % 2 == 0:
                # halves padded to one full PSUM bank each: an accumulating
                # matmul region must not straddle a bank boundary
                pstate[v] = psv.tile([P, 2, 512], F32, tag="vp",
                                     name="v_psp")
            v_psp = pstate[v - v % 2]
            for j in range(NT):
                nc.tensor.matmul(v_psp[:, v % 2, j * W:(j + 1) * W],
                                 xt_t[:, j * P:(j + 1) * P],
                                 wva_t[:], start=True, stop=True)
            state[v] = (xt_t, zt_sb)

            if v % 2 == 1 or v == n_v - 1:
                v0 = v - v % 2
                hi = v % 2 + 1
                vpw = v_psp[:, :, 0:NT * W].rearrange(
                    "p u (c w) -> p u c w", w=W)
                # eb = exp(beta), ea = exp(-alpha), paired
                ebea = sbs.tile([P, 2, NT, 2], F16, tag="ebea", name="ebea")
                nc.scalar.activation(ebea[:, 0:hi], vpw[:, 0:hi, :, DV:DV + 2],
                                     AF.Exp)
                # v4 = (vproj + bv) * eb ; ones-col holds eb
                v4p = sbv.tile([P, 2, NT, DV + 1], F16, tag="v4", name="v4p")
                nc.vector.tensor_add(
                    v4p[:, 0:hi, :, 0:DV], vpw[:, 0:hi, :, 0:DV],
                    bvb_t[:, None, None, :].broadcast_to([P, hi, NT, DV]))
                nc.gpsimd.tensor_mul(
                    v4p[:, 0:hi, :, 0:DV], v4p[:, 0:hi, :, 0:DV],
                    ebea[:, 0:hi, :, 0:1].broadcast_to([P, hi, NT, DV]))
                nc.gpsimd.tensor_copy(v4p[:, 0:hi, :, DV],
                                      ebea[:, 0:hi, :, 0])
                pstate[v0] = (v_psp, v4p, ebea)

          def mid(v):
            xt_t, zt_sb = state[v]
            et = sbe.tile([P, SC_TOT], F16, tag="et", name="et")
            eti = et[:].bitcast(I16)

            # segment A: mask (constant strictly-triangular -24, one
            # matmul via tiled identity) + the 4 diagonal blocks -> Act exp
            sa = pss.tile([P, 512], F32, tag="sgA", name="sa")
            nc.tensor.matmul(sa[:], trit_t[:], i4_t[:], start=True,
                             stop=False)
            for j in range(NT):
                nc.tensor.matmul(sa[:, j * P:(j + 1) * P],
                                 xt_t[:, j * P:(j + 1) * P],
                                 zt_sb[:, j * P:(j + 1) * P],
                                 start=False, stop=True,
                                 skip_group_check=True)
            nc.scalar.activation(et[:, 0:512], sa[:], AF.Exp)

            # segment BC: off0 (384) + off2 (128) + off1 (256) in one
            # 2-bank tile; Act exps [0:ACT_B), DVE Schraudolph the rest
            # in a single instruction.
            sb_ = pss.tile([P, 768], F32, tag="sgB", name="sb_")
            nc.tensor.matmul(sb_[:, 0:384], xt_t[:, 0:P],
                             zt_sb[:, P:T], start=True, stop=True)
            nc.tensor.matmul(sb_[:, 384:512], xt_t[:, 2 * P:3 * P],
                             zt_sb[:, 3 * P:T], start=True, stop=True)
            nc.tensor.matmul(sb_[:, 512:768], xt_t[:, P:2 * P],
                             zt_sb[:, 2 * P:T], start=True, stop=True)
            if ACT_B > 0:
                nc.scalar.activation(et[:, 512:512 + ACT_B],
                                     sb_[:, 0:ACT_B], AF.Exp)
            nc.vector.tensor_scalar(eti[:, 512 + ACT_B:1280],
                                    sb_[:, ACT_B:768],
                                    SCH_C1, SCH_C2, ALU.mult, ALU.add)
            state[v] = et

          def back(v):
            et = state.pop(v)
            E = DV + 1
            if v % 2 == 0:
                ostate[v] = pso.tile([P, 2, 512], F32, tag="op",
                                     name="pop")
            pop = ostate[v - v % 2]
            for i in range(NT):
                for j in range(i + 1):
                    if i == j:
                        loc = j * P
                    else:
                        loc = OFF_BASE[j] + (i - j - 1) * P
                    nc.tensor.matmul(
                        pop[:, v % 2, i * E:(i + 1) * E],
                        et[:, loc:loc + P],
                        pstate[v - v % 2][1][:, v % 2, j, :],
                        start=(j == 0), stop=(j == i))

            if v % 2 == 1 or v == n_v - 1:
                v0 = v - v % 2
                hi = v % 2 + 1
                _, v4p, ebea = pstate.pop(v0)
                po4 = ostate.pop(v0)[:, :, 0:NT * E].rearrange(
                    "p u (i e) -> p u i e", e=E)
                den4 = sbs.tile([P, 2, NT], F32, tag="den", name="den4")
                nc.gpsimd.tensor_mul(den4[:, 0:hi], cnt_t[:, None, :]
                                     .broadcast_to([P, hi, NT]),
                                     ebea[:, 0:hi, :, 1])
                nc.vector.tensor_add(den4[:, 0:hi], den4[:, 0:hi],
                                     po4[:, 0:hi, :, DV])
                rec4 = sbs.tile([P, 2, NT], F32, tag="rec", name="rec4")
                nc.vector.reciprocal(rec4[:, 0:hi], den4[:, 0:hi])
                ofp = sbo.tile([P, 2, NT, DV], F16, tag="of", name="ofp")
                nc.vector.tensor_mul(
                    ofp[:, 0:hi], po4[:, 0:hi, :, 0:DV],
                    rec4[:, 0:hi, :, None].broadcast_to([P, hi, NT, DV]))
                nc.sync.dma_start(
                    out[v0:v0 + hi].rearrange("v p (i e) -> p v i e", e=DV),
                    ofp[:, 0:hi])

          for k in range(n_v + 2):
            if k < n_v:
                front(k)
            if 0 <= k - 1 < n_v:
                mid(k - 1)
            if 0 <= k - 2 < n_v:
                back(k - 2)

    nc.compile()
    return nc


def get_program(n_v=V, n_rep=1):
    key = (n_v, n_rep)
    if key not in _PROGRAM_CACHE:
        _PROGRAM_CACHE[key] = build_program(n_v, n_rep)
    return _PROGRAM_CACHE[key]


def host_inputs(x, Wq, bq, Wk, bk, Wv, bv):
    """Build the per-core input maps (host-side data staging)."""
    x = np.asarray(x, dtype=np.float32)
    Wq = np.asarray(Wq, dtype=np.float32)
    bq = np.asarray(bq, dtype=np.float32)
    Wk = np.asarray(Wk, dtype=np.float32)
    bk = np.asarray(bk, dtype=np.float32)
    Wv = np.asarray(Wv, dtype=np.float32)
    bv = np.asarray(bv, dtype=np.float32)

    scale = np.float64(1.0) / np.sqrt(np.float64(DK))
    Wq64 = Wq.astype(np.float64)
    Wk64 = Wk.astype(np.float64)
    # A = scale * Wq^T Wk; device z = A^T x so scoresT[s,t] = x_s . z_t
    amh = np.ascontiguousarray(scale * (Wq64.T @ Wk64)).astype(np.float16)
    # bias cross-terms: alpha[t] = w_a . x_t, beta[s] = w_b . x_s
    w_a = scale * (Wq64.T @ bk.astype(np.float64))   # (C,)
    w_b = scale * (Wk64.T @ bq.astype(np.float64))   # (C,)
    c0 = float(scale * np.dot(bq.astype(np.float64), bk.astype(np.float64)))
    wvah = np.ascontiguousarray(np.concatenate(
        [Wv.T.astype(np.float64), w_b[:, None], -w_a[:, None]],
        axis=1)).astype(np.float16)                  # (C, DV+2)
    bvbh = np.ascontiguousarray(
        np.broadcast_to(bv, (P, DV))).astype(np.float32)

    tl = np.arange(P, dtype=np.int64)
    ii = np.arange(NT, dtype=np.int64)
    cnth = (((T - 1) - (ii[None, :] * P + tl[:, None])) *
            np.exp(-c0)).astype(np.float32)

    # mask matrix: trit[c, p] = -30000 where p > c (strictly upper in
    # storage); psum[p, j*128+c] += trit[c, p]
    ci = np.arange(P)
    trith = np.where(ci[None, :] > ci[:, None], np.float16(-24.0),
                     np.float16(0.0)).astype(np.float16)
    i4h = np.tile(np.eye(P, dtype=np.float16), (1, NT))

    # (B, T, V, C) -> (B, V, C, T), fp16
    xth = np.ascontiguousarray(x.transpose(0, 2, 3, 1)).astype(np.float16)

    in_maps = []
    for b in range(N_CORES):
        in_maps.append({
            "xt": xth[b],
            "am": amh, "wva": wvah, "bvb": bvbh,
            "cnt": cnth, "trit": trith, "i4": i4h,
        })
    return in_maps


def run(x, Wq, bq, Wk, bk, Wv, bv, trace=False):
    """Run on 8 cores; returns (output, BassKernelResults)."""
    nc = get_program(V)
    in_maps = host_inputs(x, Wq, bq, Wk, bk, Wv, bv)
    res = run_bass_kernel_spmd(nc, in_maps, list(range(N_CORES)), trace=trace)
    outp = np.empty((B, T, V, DV), dtype=np.float32)
    for b in range(N_CORES):
        arr = res.results[b]["out"].reshape(V, P, NT, DV)
        # out[t = i*128 + p, v, e] = arr[v, p, i, e]
        outp[b] = arr.transpose(2, 1, 0, 3).reshape(T, V, DV).astype(
            np.float32)
    return outp, res


def kernel(x, Wq, bq, Wk, bk, Wv, bv):
    outp, _ = run(x, Wq, bq, Wk, bk, Wv, bv, trace=False)
    return outp


# revision 30
# speedup vs baseline: 1.0193x; 1.0193x over previous
"""Trainium2 Bass kernel for nn_AttentionHead (B=8, T=512, V=25, C=128, Dk=Dv=64).

Sharding: data-parallel over batch B across 8 NeuronCores (batch b -> core b).
No cross-device communication.

Per-core design (per vertex v, fp16 datapath, fp32 PSUM accumulation):
- x pre-transposed on host to (V, C, T) fp16; one contiguous DMA per
  vertex pair in, one per pair out (512B+ lines both directions).
- scoresT ([s, t]) layout packs the 4 diagonal 128x128 blocks adjacently at
  cols [0:512), followed by the off-diagonal groups:
  off0 (s-chunk0, t=128..511) at [512:896), off1 at [896:1152),
  off2 at [1152:1280).
- causal masking costs no vector work: a constant strictly-triangular
  -24 matrix is accumulated into the diagonal blocks by one extra
  128->512 matmul (lhsT=trit, rhs=tiled identity); exp(-24+s) < 4e-10
  underflows to 0 in fp16 (and is negligible even in exact arithmetic).
- exp is split across engines: Act table-exps the diagonal segment (all
  masked cols) plus the first ACT_B off-diag cols; DVE approximates exp
  on the never-masked remainder via the Schraudolph bit trick:
  et_i16 = round(s*1477.32 + 15308) bitcast fp16 (rel err ~8e-3).
- value matrix carries an eb=exp(beta) column so the out matmul
  accumulates the softmax denominator for free; bias cross terms
  (alpha_t, beta_s) folded via the augmented wva projection.
- unoccupied slots contribute exp(0)=1: den += (511-t)*exp(-c0-alpha_t),
  via the host cnt table and ea=exp(-alpha).
- elementwise work is paired across two vertices (half the instruction
  overhead). GPSIMD cannot touch PSUM, so all PSUM->SBUF crossings run
  on Act (z copy, diag exp) and DVE (Schraudolph, v4 add, den add,
  normalize); GpSimd handles SBUF-only f16 ops (eb scale, ones column,
  den cnt*ea). PSUM pair tiles pad each half to a full 2KB bank -- an
  accumulating matmul region must not straddle a bank boundary.
"""

import numpy as np
from contextlib import ExitStack

import concourse.bass as bass  # noqa: F401
import concourse.tile as tile
from concourse import bacc, mybir
from concourse.bass_utils import run_bass_kernel_spmd

B, T, V, C = 8, 512, 25, 128
DK, DV = 64, 64
P = 128
NT = T // P  # 4 tiles of 128 along T
N_CORES = 8

# et column layout: 4 diag blocks at [0:512), then off-diag groups:
# off0 (s-chunk0, 384 cols) at [512:896), off2 (128) at [896:1024),
# off1 (256) at [1024:1280). PSUM is processed as three 512-col
# segments: A=[0:512) diag, B=[512:1024) off0+off2, C=[1024:1280) off1.
OFF_BASE = {0: 512, 1: 1024, 2: 896}
SC_TOT = 1280

# Schraudolph fp16 exp constants (tuned numerically: rel err ~7.8e-3)
SCH_C1 = 1024.0 / float(np.log(2.0))
SCH_C2 = 15.0 * 1024.0 - 52.0

# engine split knobs (sim-tuned): Act additionally exps segB[0:ACT_B);
# z-copy cols [0:ZA) on Act, rest on DVE.
ACT_B = 192
ZA = 512

F32 = mybir.dt.float32
F16 = mybir.dt.float16
I16 = mybir.dt.int16
AF = mybir.ActivationFunctionType
ALU = mybir.AluOpType

_PROGRAM_CACHE = {}


def build_program(n_v=V, n_rep=1):
    nc = bacc.Bacc(
        "TRN2", target_bir_lowering=False, debug=False, num_devices=N_CORES
    )
    xt = nc.dram_tensor("xt", [n_v, C, T], F16, kind="ExternalInput").ap()
    am = nc.dram_tensor("am", [C, C], F16, kind="ExternalInput").ap()
    wva = nc.dram_tensor("wva", [C, DV + 2], F16, kind="ExternalInput").ap()
    bvb = nc.dram_tensor("bvb", [P, DV], F32, kind="ExternalInput").ap()
    cnt = nc.dram_tensor("cnt", [P, NT], F32, kind="ExternalInput").ap()
    trit = nc.dram_tensor("trit", [P, P], F16, kind="ExternalInput").ap()
    i4 = nc.dram_tensor("i4", [P, NT * P], F16, kind="ExternalInput").ap()
    out = nc.dram_tensor("out", [n_v, P, NT * DV], F16,
                         kind="ExternalOutput").ap()

    with tile.TileContext(nc) as tc, ExitStack() as ctx:
        consts = ctx.enter_context(tc.tile_pool(name="consts", bufs=1))
        sbx = ctx.enter_context(tc.tile_pool(name="sbx", bufs=3))
        sbqk = ctx.enter_context(tc.tile_pool(name="sbqk", bufs=3))
        sbv = ctx.enter_context(tc.tile_pool(name="sbv", bufs=5))
        sbe = ctx.enter_context(tc.tile_pool(name="sbe", bufs=3))
        sbo = ctx.enter_context(tc.tile_pool(name="sbo", bufs=2))
        sbs = ctx.enter_context(tc.tile_pool(name="sbs", bufs=8))
        psq = ctx.enter_context(tc.tile_pool(name="psq", bufs=1, space="PSUM"))
        psv = ctx.enter_context(tc.tile_pool(name="psv", bufs=1, space="PSUM"))
        pso = ctx.enter_context(tc.tile_pool(name="pso", bufs=1, space="PSUM"))
        pss = ctx.enter_context(tc.tile_pool(name="pss", bufs=1, space="PSUM"))

        am_t = consts.tile([C, C], F16)
        nc.sync.dma_start(am_t[:], am[:])
        wva_t = consts.tile([C, DV + 2], F16)
        nc.sync.dma_start(wva_t[:], wva[:])
        bvb_t = consts.tile([P, DV], F32)
        nc.sync.dma_start(bvb_t[:], bvb[:])
        cnt_t = consts.tile([P, NT], F32)
        nc.sync.dma_start(cnt_t[:], cnt[:])
        trit_t = consts.tile([P, P], F16)
        nc.sync.dma_start(trit_t[:], trit[:])
        i4_t = consts.tile([P, NT * P], F16)
        nc.sync.dma_start(i4_t[:], i4[:])

        for rep in range(n_rep):
          xt_pair = {}
          state = {}
          pstate = {}
          ostate = {}

          def front(v):
            if v % 2 == 0:
                xp = sbx.tile([C, 2, T], F16, tag="xt", name="xp")
                hi = min(2, n_v - v)
                nc.sync.dma_start(
                    xp[:, 0:hi, :],
                    xt[v:v + hi].rearrange("v c t -> c v t"))
                xt_pair[v] = xp
            xt_t = xt_pair[v - v % 2][:, v % 2, :]

            # z = A^T x (A = scale * Wq^T Wk folded on host)
            z_ps = psq.tile([C, T], F32, tag="zp", name="z_ps")
            nc.tensor.matmul(z_ps[:], am_t[:], xt_t, start=True, stop=True)
            zt_sb = sbqk.tile([C, T], F16, tag="zts", name="zt_sb")
            if ZA > 0:
                nc.scalar.activation(zt_sb[:, 0:ZA], z_ps[:, 0:ZA], AF.Copy)
            if ZA < T:
                nc.vector.tensor_copy(zt_sb[:, ZA:T], z_ps[:, ZA:T])

            # v projection into the pair slot: [p, 2, NT*(DV+2)]
            W = DV + 2
            if v % 2 == 0:
                # halves padded to one full PSUM bank each: an accumulating
                # matmul region must not straddle a bank boundary
                pstate[v] = psv.tile([P, 2, 512], F32, tag="vp",
                                     name="v_psp")
            v_psp = pstate[v - v % 2]
            for j in range(NT):
                nc.tensor.matmul(v_psp[:, v % 2, j * W:(j + 1) * W],
                                 xt_t[:, j * P:(j + 1) * P],
                                 wva_t[:], start=True, stop=True)
            state[v] = (xt_t, zt_sb)

            if v % 2 == 1 or v == n_v - 1:
                v0 = v - v % 2
                hi = v % 2 + 1
                vpw = v_psp[:, :, 0:NT * W].rearrange(
                    "p u (c w) -> p u c w", w=W)
                # eb = exp(beta), ea = exp(-alpha), paired
                ebea = sbs.tile([P, 2, NT, 2], F16, tag="ebea", name="ebea")
                nc.scalar.activation(ebea[:, 0:hi], vpw[:, 0:hi, :, DV:DV + 2],
                                     AF.Exp)
                # v4 = (vproj + bv) * eb ; ones-col holds eb
                v4p = sbv.tile([P, 2, NT, DV + 1], F16, tag="v4", name="v4p")
                nc.vector.tensor_add(
                    v4p[:, 0:hi, :, 0:DV], vpw[:, 0:hi, :, 0:DV],
                    bvb_t[:, None, None, :].broadcast_to([P, hi, NT, DV]))
                nc.gpsimd.tensor_mul(
                    v4p[:, 0:hi, :, 0:DV], v4p[:, 0:hi, :, 0:DV],
                    ebea[:, 0:hi, :, 0:1].broadcast_to([P, hi, NT, DV]))
                nc.gpsimd.tensor_copy(v4p[:, 0:hi, :, DV],
                                      ebea[:, 0:hi, :, 0])
                pstate[v0] = (v_psp, v4p, ebea)

          def mid(v):
            xt_t, zt_sb = state[v]
            et = sbe.tile([P, SC_TOT], F16, tag="et", name="et")
            eti = et[:].bitcast(I16)

            # segment A: mask (constant strictly-triangular -24, one
            # matmul via tiled identity) + the 4 diagonal blocks -> Act exp
            sa = pss.tile([P, 512], F32, tag="sgA", name="sa")
            nc.tensor.matmul(sa[:], trit_t[:], i4_t[:], start=True,
                             stop=False)
            for j in range(NT):
                nc.tensor.matmul(sa[:, j * P:(j + 1) * P],
                                 xt_t[:, j * P:(j + 1) * P],
                                 zt_sb[:, j * P:(j + 1) * P],
                                 start=False, stop=True,
                                 skip_group_check=True)
            nc.scalar.activation(et[:, 0:512], sa[:], AF.Exp)

            # segment BC: off0 (384) + off2 (128) + off1 (256) in one
            # 2-bank tile; Act exps [0:ACT_B), DVE Schraudolph the rest
            # in a single instruction.
            sb_ = pss.tile([P, 768], F32, tag="sgB", name="sb_")
            nc.tensor.matmul(sb_[:, 0:384], xt_t[:, 0:P],
                             zt_sb[:, P:T], start=True, stop=True)
            nc.tensor.matmul(sb_[:, 384:512], xt_t[:, 2 * P:3 * P],
                             zt_sb[:, 3 * P:T], start=True, stop=True)
            nc.tensor.matmul(sb_[:, 512:768], xt_t[:, P:2 * P],
                             zt_sb[:, 2 * P:T], start=True, stop=True)
            if ACT_B > 0:
                nc.scalar.activation(et[:, 512:512 + ACT_B],
                                     sb_[:, 0:ACT_B], AF.Exp)
            nc.vector.tensor_scalar(eti[:, 512 + ACT_B:1280],
                                    sb_[:, ACT_B:768],
                                    SCH_C1, SCH_C2, ALU.mult, ALU.add)
            state[v] = et

          def back(v):
            et = state.pop(v)
            E = DV + 1
            if v % 2 == 0:
                ostate[v] = pso.tile([P, 2, 512], F32, tag="op",
                                     name="pop")
            pop = ostate[v - v % 2]
            for i in range(NT):
                for j in range(i + 1):
                    if i == j:
                        loc = j * P
                    else:
                        loc = OFF_BASE[j] + (i - j - 1) * P
                    nc.tensor.matmul(
                        pop[:, v % 2, i * E:(i + 1) * E],
                        et[:, loc:loc + P],
                        pstate[v - v % 2][1][:, v % 2, j, :],
                        start=(j == 0), stop=(j == i))

            if v % 2 == 1 or v == n_v - 1:
                v0 = v - v % 2
                hi = v % 2 + 1
                _, v4p, ebea = pstate.pop(v0)
                po4 = ostate.pop(v0)[:, :, 0:NT * E].rearrange(
                    "p u (i e) -> p u i e", e=E)
                den4 = sbs.tile([P, 2, NT], F32, tag="den", name="den4")
                nc.gpsimd.tensor_mul(den4[:, 0:hi], cnt_t[:, None, :]
                                     .broadcast_to([P, hi, NT]),
                                     ebea[:, 0:hi, :, 1])
                nc.vector.tensor_add(den4[:, 0:hi], den4[:, 0:hi],
                                     po4[:, 0:hi, :, DV])
                rec4 = sbs.tile([P, 2, NT], F32, tag="rec", name="rec4")
                nc.vector.reciprocal(rec4[:, 0:hi], den4[:, 0:hi])
                ofp = sbo.tile([P, 2, NT, DV], F16, tag="of", name="ofp")
                nc.vector.tensor_mul(
                    ofp[:, 0:hi], po4[:, 0:hi, :, 0:DV],
                    rec4[:, 0:hi, :, None].broadcast_to([P, hi, NT, DV]))
                nc.sync.dma_start(
                    out[v0:v0 + hi].rearrange("v p (i e) -> p v i e", e=DV),
                    ofp[:, 0:hi])

          for k in range(n_v + 2):
            if k < n_v:
                front(k)
            if 0 <= k - 1 < n_v:
                mid(k - 1)
            if 0 <= k - 2 < n_v:
                back(k - 2)

    nc.compile()
    return nc


def get_program(n_v=V, n_rep=1):
    key = (n_v, n_rep)
    if key not in _PROGRAM_CACHE:
        _PROGRAM_CACHE[key] = build_program(n_v, n_rep)
    return _PROGRAM_CACHE[key]


def host_inputs(x, Wq, bq, Wk, bk, Wv, bv):
    """Build the per-core input maps (host-side data staging)."""
    x = np.asarray(x, dtype=np.float32)
    Wq = np.asarray(Wq, dtype=np.float32)
    bq = np.asarray(bq, dtype=np.float32)
    Wk = np.asarray(Wk, dtype=np.float32)
    bk = np.asarray(bk, dtype=np.float32)
    Wv = np.asarray(Wv, dtype=np.float32)
    bv = np.asarray(bv, dtype=np.float32)

    scale = np.float64(1.0) / np.sqrt(np.float64(DK))
    Wq64 = Wq.astype(np.float64)
    Wk64 = Wk.astype(np.float64)
    # A = scale * Wq^T Wk; device z = A^T x so scoresT[s,t] = x_s . z_t
    amh = np.ascontiguousarray(scale * (Wq64.T @ Wk64)).astype(np.float16)
    # bias cross-terms: alpha[t] = w_a . x_t, beta[s] = w_b . x_s
    w_a = scale * (Wq64.T @ bk.astype(np.float64))   # (C,)
    w_b = scale * (Wk64.T @ bq.astype(np.float64))   # (C,)
    c0 = float(scale * np.dot(bq.astype(np.float64), bk.astype(np.float64)))
    wvah = np.ascontiguousarray(np.concatenate(
        [Wv.T.astype(np.float64), w_b[:, None], -w_a[:, None]],
        axis=1)).astype(np.float16)                  # (C, DV+2)
    bvbh = np.ascontiguousarray(
        np.broadcast_to(bv, (P, DV))).astype(np.float32)

    tl = np.arange(P, dtype=np.int64)
    ii = np.arange(NT, dtype=np.int64)
    cnth = (((T - 1) - (ii[None, :] * P + tl[:, None])) *
            np.exp(-c0)).astype(np.float32)

    # mask matrix: trit[c, p] = -30000 where p > c (strictly upper in
    # storage); psum[p, j*128+c] += trit[c, p]
    ci = np.arange(P)
    trith = np.where(ci[None, :] > ci[:, None], np.float16(-24.0),
                     np.float16(0.0)).astype(np.float16)
    i4h = np.tile(np.eye(P, dtype=np.float16), (1, NT))

    # (B, T, V, C) -> (B, V, C, T), fp16
    xth = np.ascontiguousarray(x.transpose(0, 2, 3, 1)).astype(np.float16)

    in_maps = []
    for b in range(N_CORES):
        in_maps.append({
            "xt": xth[b],
            "am": amh, "wva": wvah, "bvb": bvbh,
            "cnt": cnth, "trit": trith, "i4": i4h,
        })
    return in_maps


def run(x, Wq, bq, Wk, bk, Wv, bv, trace=False):
    """Run on 8 cores; returns (output, BassKernelResults)."""
    nc = get_program(V)
    in_maps = host_inputs(x, Wq, bq, Wk, bk, Wv, bv)
    res = run_bass_kernel_spmd(nc, in_maps, list(range(N_CORES)), trace=trace)
    outp = np.empty((B, T, V, DV), dtype=np.float32)
    for b in range(N_CORES):
        arr = res.results[b]["out"].reshape(V, P, NT, DV)
        # out[t = i*128 + p, v, e] = arr[v, p, i, e]
        outp[b] = arr.transpose(2, 1, 0, 3).reshape(T, V, DV).astype(
            np.float32)
    return outp, res


def kernel(x, Wq, bq, Wk, bk, Wv, bv):
    outp, _ = run(x, Wq, bq, Wk, bk, Wv, bv, trace=False)
    return outp
